# revision 1
# baseline (speedup 1.0000x reference)
"""Trainium2 Bass kernel for the CSDGI encoder/decoder (2-cluster graph message passing).

Data-parallel over batch: B=512 -> 64 rows per core on 8 NeuronCores.

Per-core dataflow (all matmuls bf16 operands, fp32 PSUM accumulate):
  encoder  : [channel, b*m] layout, residual folded into weights (W+I),
             biases via per-partition ACT bias, 3 small TensorE matmuls/chunk
  transpose: E [96, b*1024+m] -> XT[m_tile][:, b*96+p] via DMA xbar (bf16)
  graph G  : on-device: row-normalize gw, gram wn^T wn, |.|, row-norm (ACT
             accum), diag zero (gpsimd affine_select)
  main     : Y[bp, n] = sum_m XT[m, bp] G[m, n] (+ gb via K=1 ones matmul)
  tanh     : ACT Tanh(0.1 * psum) -> bf16
  * fw     : VectorE tensor_mul with pre-tiled fw pattern (period-3 over tiles)
  sum_p    : one-hot selector matmul on TensorE (+ fb via K=1 matmul)
"""

import numpy as np
import ml_dtypes

import concourse.bass as bass
from concourse import bacc
import concourse.mybir as mybir
from concourse.tile import TileContext

BF16 = mybir.dt.bfloat16
F32 = mybir.dt.float32

B, M, P, CH = 512, 1024, 96, 32
NCORES = 8
BL = B // NCORES          # 64 batch rows per core
CB = 8                    # batch rows per chunk
NCHUNKS = BL // CB        # 8 chunks
NMT = M // 128            # 8 m-tiles
NBP = CB * P // 128       # 6 bp-tiles of 128 per chunk
SCALE = 0.1               # 1/SCALE_ALPHA
EPS = 1e-10


def build_nc(n_chunks=NCHUNKS):
    nc = bacc.Bacc(None, target_bir_lowering=False)

    # ---- DRAM I/O ----
    x_d = nc.dram_tensor("x", [NCHUNKS, CB * M], BF16, kind="ExternalInput")
    a0_d = nc.dram_tensor("a0", [1, CH], BF16, kind="ExternalInput")
    w1_d = nc.dram_tensor("w1t", [CH, CH], BF16, kind="ExternalInput")
    w2_d = nc.dram_tensor("w2t", [2 * CH, CH], BF16, kind="ExternalInput")
    encb_d = nc.dram_tensor("encb", [P, 1], F32, kind="ExternalInput")
    gw_d = [nc.dram_tensor(f"gw{c}", [64, M], F32, kind="ExternalInput") for c in range(2)]
    gb_d = [nc.dram_tensor(f"gbb{c}", [1, M], BF16, kind="ExternalInput") for c in range(2)]
    fb_d = [nc.dram_tensor(f"fbb{c}", [1, M], BF16, kind="ExternalInput") for c in range(2)]
    fw_d = [nc.dram_tensor(f"fwB{c}", [3 * 128, M], BF16, kind="ExternalInput") for c in range(2)]
    s8_d = nc.dram_tensor("s8", [NBP * 128, CB], BF16, kind="ExternalInput")
    ones_d = nc.dram_tensor("ones", [1, 128], BF16, kind="ExternalInput")
    ident_d = nc.dram_tensor("ident", [P, P], BF16, kind="ExternalInput")
    out_d = [nc.dram_tensor(f"out{c}", [BL, M], F32, kind="ExternalOutput") for c in range(2)]

    with TileContext(nc) as tc:
        with (
            tc.tile_pool(name="const", bufs=1) as cst,
            tc.tile_pool(name="setup", bufs=2) as stp,
            tc.tile_pool(name="work", bufs=2) as wrk,
            tc.tile_pool(name="tt", bufs=3) as ttp,
            tc.tile_pool(name="psA", bufs=2, space="PSUM") as psA,
            tc.tile_pool(name="psY", bufs=2, space="PSUM") as psY,
            tc.tile_pool(name="psO", bufs=1, space="PSUM") as psO,
            tc.tile_pool(name="psT", bufs=2, space="PSUM") as psT,
        ):
            # ---- constants into SBUF ----
            gww_s = [stp.tile([64, M], F32, tag=f"gww{c}", name=f"gww{c}")
                     for c in range(2)]
            for c in range(2):
                nc.gpsimd.dma_start(gww_s[c], gw_d[c][:, :])
            a0_s = cst.tile([1, CH], BF16, tag="a0", name="a0")
            nc.gpsimd.dma_start(a0_s, a0_d[:, :])
            w1_s = cst.tile([CH, CH], BF16, tag="w1", name="w1")
            nc.gpsimd.dma_start(w1_s, w1_d[:, :])
            w2_s = cst.tile([2 * CH, CH], BF16, tag="w2", name="w2")
            nc.gpsimd.dma_start(w2_s, w2_d[:, :])
            encb_s = cst.tile([P, 1], F32, tag="encb", name="encb")
            nc.gpsimd.dma_start(encb_s, encb_d[:, :])
            ones_s = cst.tile([1, 128], BF16, tag="ones", name="ones")
            nc.gpsimd.dma_start(ones_s, ones_d[:, :])
            ident_s = cst.tile([P, P], BF16, tag="ident", name="ident")
            nc.gpsimd.dma_start(ident_s, ident_d[:, :])
            gb_s, fb_s, fw_s, s8_s = [], [], [], []
            for c in range(2):
                t = cst.tile([1, M], BF16, tag=f"gb{c}", name=f"gb{c}")
                nc.gpsimd.dma_start(t, gb_d[c][:, :])
                gb_s.append(t)
                t = cst.tile([1, M], BF16, tag=f"fb{c}", name=f"fb{c}")
                nc.gpsimd.dma_start(t, fb_d[c][:, :])
                fb_s.append(t)
                fr = []
                for r in range(3):
                    t = cst.tile([128, M], BF16, tag=f"fw{c}_{r}", name=f"fw{c}_{r}")
                    nc.gpsimd.dma_start(t, fw_d[c][r * 128:(r + 1) * 128, :])
                    fr.append(t)
                fw_s.append(fr)
            for i in range(NBP):
                t = cst.tile([128, CB], BF16, tag=f"s8_{i}", name=f"s8_{i}")
                nc.gpsimd.dma_start(t, s8_d[i * 128:(i + 1) * 128, :])
                s8_s.append(t)

            # ---- G matrices (per cluster), stored bf16 [128, 1024] x 8 tiles
            # Emitted as weavable units so PE can interleave the gram matmuls
            # with the chunk-0/1 encoder prologue.
            G_s = [[cst.tile([128, M], BF16, tag=f"G{c}_{mt}", name=f"G{c}_{mt}") for mt in range(NMT)]
                   for c in range(2)]
            wnb_s = [cst.tile([64, M], BF16, tag=f"wnb{c}", name=f"wnb{c}")
                     for c in range(2)]

            def setup_norm(c):
                def emit():
                    gww = gww_s[c]
                    sq = stp.tile([64, M], F32, tag="sq", name="sq")
                    ss = stp.tile([64, 1], F32, tag="ss", name="ss")
                    nc.scalar.activation(sq, gww,
                                         mybir.ActivationFunctionType.Square,
                                         accum_out=ss)
                    nrm = stp.tile([64, 1], F32, tag="nrm", name="nrm")
                    nc.scalar.activation(nrm, ss,
                                         mybir.ActivationFunctionType.Sqrt)
                    nc.vector.tensor_scalar_max(nrm, nrm, EPS)
                    rinv = stp.tile([64, 1], F32, tag="rinv", name="rinv")
                    nc.vector.reciprocal(rinv, nrm)
                    nc.scalar.activation(wnb_s[c], gww,
                                         mybir.ActivationFunctionType.Copy,
                                         scale=rinv)
                return emit

            def setup_gmt(c, mt):
                def emit():
                    wnb = wnb_s[c]
                    gf = stp.tile([128, M], F32, tag="gf", name="gf")
                    rs = [stp.tile([128, 1], F32, tag=f"rs{h}", name=f"rs{h}") for h in range(2)]
                    for h in range(2):
                        pg = psY.tile([128, 512], F32, tag="pY", name="pY")
                        nc.tensor.matmul(pg, lhsT=wnb[:, mt * 128:(mt + 1) * 128],
                                         rhs=wnb[:, h * 512:(h + 1) * 512],
                                         start=True, stop=True)
                        nc.scalar.activation(gf[:, h * 512:(h + 1) * 512], pg,
                                             mybir.ActivationFunctionType.Abs,
                                             accum_out=rs[h])
                    rsum = stp.tile([128, 1], F32, tag="rsum", name="rsum")
                    nc.vector.tensor_add(rsum, rs[0], rs[1])
                    rsinv = stp.tile([128, 1], F32, tag="rsinv", name="rsinv")
                    nc.vector.reciprocal(rsinv, rsum)
                    nc.scalar.activation(G_s[c][mt], gf,
                                         mybir.ActivationFunctionType.Copy,
                                         scale=rsinv)
                    # zero diagonal: keep where (mt*128 + p - j) != 0
                    nc.gpsimd.affine_select(
                        out=G_s[c][mt], in_=G_s[c][mt],
                        compare_op=mybir.AluOpType.not_equal, fill=0.0,
                        base=mt * 128, channel_multiplier=1, pattern=[[-1, M]])
                return emit

            setup_units = [setup_norm(0)] + [setup_gmt(0, mt) for mt in range(NMT)] \
                + [setup_norm(1)] + [setup_gmt(1, mt) for mt in range(NMT)]

            # ---- main pipeline over batch chunks (software-pipelined by 1) ----
            def evict_relu(k, dst, src, bias):
                # balance PSUM->SBUF relu evictions between ACT and DVE
                if k % 8 < 3:
                    nc.scalar.activation(dst, src,
                                         mybir.ActivationFunctionType.Relu,
                                         bias=bias)
                else:
                    nc.vector.tensor_scalar(dst, src, bias, 0.0,
                                            op0=mybir.AluOpType.add,
                                            op1=mybir.AluOpType.max)

            def encoder_steps(cb):
                """Return (XT, fillers): fine-grained emission callables, one
                matmul+eviction (or one b's transpose batch) each."""
                xr = wrk.tile([1, CB * M], BF16, tag="xr", name="xr", bufs=2)
                nc.gpsimd.dma_start(xr, x_d[cb:cb + 1, :])
                E = wrk.tile([P, CB * M], BF16, tag="E", name="E", bufs=2)
                XT = [wrk.tile([128, CB * P], BF16, tag=f"XT{mt}",
                               name=f"XT{mt}", bufs=3) for mt in range(NMT)]
                pEs = {}

                def sub(j, layer):
                    def emit():
                        sl = bass.ts(j, 512)
                        if layer == 0:
                            pEs[j] = psA.tile([P, 512], F32, tag="pE", name="pE")
                        pE = pEs[j]
                        lo, hi = 32 * layer, 32 * layer + 32
                        if layer == 0:
                            nc.tensor.matmul(pE[0:32], lhsT=a0_s, rhs=xr[:, sl],
                                             start=True, stop=True,
                                             tile_position=(0, 0))
                        elif layer == 1:
                            nc.tensor.matmul(pE[32:64], lhsT=w1_s,
                                             rhs=E[0:32, sl], start=True,
                                             stop=True, tile_position=(0, 32))
                        else:
                            nc.tensor.matmul(pE[64:96], lhsT=w2_s[32:64],
                                             rhs=E[32:64, sl], start=True,
                                             stop=True, tile_position=(32, 64))
                        evict_relu(3 * j + layer, E[lo:hi, sl], pE[lo:hi],
                                   encb_s[lo:hi])
                    return emit

                def transposes(b):
                    def emit():
                        for mt in range(NMT):
                            k = b * NMT + mt
                            esl = E[:, b * M + mt * 128: b * M + (mt + 1) * 128]
                            xsl = XT[mt][:, b * P:(b + 1) * P]
                            if k % 2 == 0:
                                # xbar DMA transpose, alternating HWDGE queues
                                eng = nc.sync if (k // 2) % 2 == 0 else nc.scalar
                                eng.dma_start_transpose(xsl, esl)
                            else:
                                # TensorE transpose + PSUM eviction
                                pt = psT.tile([128, P], BF16, tag="pT", name="pT")
                                nc.tensor.transpose(pt, esl, ident_s)
                                if k % 4 == 1:
                                    nc.scalar.activation(
                                        xsl, pt,
                                        mybir.ActivationFunctionType.Copy)
                                else:
                                    nc.vector.tensor_copy(xsl, pt)
                    return emit

                steps = []
                for b in range(CB):
                    for j in (2 * b, 2 * b + 1):
                        steps += [sub(j, 0), sub(j, 1), sub(j, 2)]
                    steps.append(transposes(b))
                return XT, steps

            def emit_main(cb, XT, fillers):
                """Emit the 24 main groups for chunk cb, injecting encoder
                filler sub-steps between matmuls so PE never stalls on the
                encoder's serial mm->evict chain."""
                fillers = list(fillers)

                def fill(k=1):
                    for _ in range(k):
                        if fillers:
                            fillers.pop(0)()

                pO = {}
                # two-level deferral so no engine's queue head waits cross-
                # engine: the fw-multiply (DVE) runs one group after its tanh
                # (ACT), the selector matmul (PE) two groups after.
                pend_mult = []   # (c, i, h, T)
                pend_sel = []    # (c, i, h, T2)

                def emit_mult():
                    c, i, h, T = pend_mult.pop(0)
                    nsl = bass.ts(h, 512)
                    T2 = ttp.tile([128, 512], BF16, tag="T2", name="T2")
                    nc.vector.tensor_mul(T2, T, fw_s[c][i % 3][:, nsl])
                    pend_sel.append((c, i, h, T2))

                def emit_sel():
                    c, i, h, T2 = pend_sel.pop(0)
                    nc.tensor.matmul(pO[c][h], lhsT=s8_s[i], rhs=T2,
                                     start=(i == 0), stop=False)

                for c in range(2):
                    pO[c] = [psO.tile([CB, 512], F32, tag=f"pO{h}",
                                      name=f"pO{h}") for h in range(2)]
                    for i in range(NBP):
                        for h in range(2):
                            nsl = bass.ts(h, 512)
                            pYt = psY.tile([128, 512], F32, tag="pY", name="pY")
                            for mt in range(NMT):
                                nc.tensor.matmul(
                                    pYt, lhsT=XT[mt][:, i * 128:(i + 1) * 128],
                                    rhs=G_s[c][mt][:, nsl],
                                    start=(mt == 0), stop=False)
                                if mt in (2, 5):
                                    fill()
                            nc.tensor.matmul(pYt, lhsT=ones_s,
                                             rhs=gb_s[c][:, nsl],
                                             start=False, stop=True)
                            if len(pend_mult) >= 1:
                                emit_mult()
                            if len(pend_sel) >= 2:
                                emit_sel()
                            fill()
                            T = ttp.tile([128, 512], BF16, tag="T", name="T")
                            nc.scalar.activation(
                                T, pYt, mybir.ActivationFunctionType.Tanh,
                                scale=SCALE)
                            pend_mult.append((c, i, h, T))
                    while pend_mult:
                        emit_mult()
                    while pend_sel:
                        emit_sel()
                    osb = wrk.tile([CB, M], F32, tag="osb", name="osb")
                    for h in range(2):
                        nsl = bass.ts(h, 512)
                        nc.tensor.matmul(pO[c][h], lhsT=ones_s[:, 0:CB],
                                         rhs=fb_s[c][:, nsl],
                                         start=False, stop=True)
                        if h == 0:
                            nc.vector.tensor_copy(osb[:, nsl], pO[c][h])
                        else:
                            nc.scalar.activation(
                                osb[:, nsl], pO[c][h],
                                mybir.ActivationFunctionType.Copy)
                    nc.gpsimd.dma_start(out_d[c][cb * CB:(cb + 1) * CB, :], osb)
                fill(len(fillers))

            # encoder runs two chunks ahead of main: XT deps are tile-granular,
            # so chunk cb's main needs ALL of transposes(cb) done; 2-ahead
            # staging gives them a full block of slack
            XT_of = {}
            XT_of[0], s0 = encoder_steps(0)
            if n_chunks > 1:
                XT_of[1], s1 = encoder_steps(1)
            else:
                s1 = []
            prologue = []
            for k in range(max(len(s0), len(s1))):
                if k < len(s0):
                    prologue.append(s0[k])
                if k < len(s1):
                    prologue.append(s1[k])
            su = list(setup_units)
            for k, f in enumerate(prologue):
                f()
                if k % 6 == 5 and su:
                    su.pop(0)()
            for f in su:
                f()
            for cb in range(n_chunks):
                if cb + 2 < n_chunks:
                    XT_of[cb + 2], fillers = encoder_steps(cb + 2)
                else:
                    fillers = []
                emit_main(cb, XT_of.pop(cb), fillers)

    nc.compile()
    return nc


def _bf(x):
    return np.asarray(x, dtype=np.float32).astype(ml_dtypes.bfloat16)


def prep_params(inputs):
    """Host-side layout/dtype prep of the small replicated parameters."""
    w0 = np.asarray(inputs["w0"], dtype=np.float32)   # [32, 1]
    w1 = np.asarray(inputs["w1"], dtype=np.float32)   # [32, 32]
    w2 = np.asarray(inputs["w2"], dtype=np.float32)
    I = np.eye(CH, dtype=np.float32)
    p = {
        "a0": _bf(w0[:, 0] + 1.0).reshape(1, CH),
        "w1t": _bf((w1 + I).T),
        "encb": np.concatenate([np.asarray(inputs[f"b{k}"], dtype=np.float32)
                                for k in range(3)]).reshape(P, 1),
    }
    w2t = np.zeros((2 * CH, CH), dtype=ml_dtypes.bfloat16)
    w2t[CH:] = _bf((w2 + I).T)
    p["w2t"] = w2t
    for c in range(2):
        p[f"gw{c}"] = np.asarray(inputs[f"gw{c}"], dtype=np.float32)
        p[f"gbb{c}"] = _bf(inputs[f"gb{c}"]).reshape(1, M)
        p[f"fbb{c}"] = _bf(np.asarray(inputs[f"fb{c}"])[:, 0]).reshape(1, M)
        F = _bf(np.asarray(inputs[f"fw{c}"])[:, :, 0].T)       # [96, M]
        fwB = np.zeros((3 * 128, M), dtype=ml_dtypes.bfloat16)
        for r in range(3):
            fwB[r * 128:(r + 1) * 128] = F[(32 * r + np.arange(128)) % P]
        p[f"fwB{c}"] = fwB
    s8 = np.zeros((NBP * 128, CB), dtype=np.float32)
    for i in range(NBP):
        for k in range(128):
            s8[i * 128 + k, (i * 128 + k) // P] = 1.0
    p["s8"] = s8.astype(ml_dtypes.bfloat16)
    p["ones"] = np.ones((1, 128), dtype=ml_dtypes.bfloat16)
    p["ident"] = np.eye(P, dtype=ml_dtypes.bfloat16)
    return p


_NC_CACHE = {}


def run(inputs, **kw):
    from concourse.bass_utils import run_bass_kernel_spmd

    if "nc" not in _NC_CACHE:
        _NC_CACHE["nc"] = build_nc()
    nc = _NC_CACHE["nc"]

    params = prep_params(inputs)
    x = np.asarray(inputs["inputs"], dtype=np.float32)[:, :, 0]   # [B, M]
    in_maps = []
    for i in range(NCORES):
        m = dict(params)
        m["x"] = _bf(x[i * BL:(i + 1) * BL]).reshape(NCHUNKS, CB * M)
        in_maps.append(m)

    res = run_bass_kernel_spmd(nc, in_maps, core_ids=list(range(NCORES)), **kw)
    y0 = np.concatenate([res.results[i]["out0"] for i in range(NCORES)], axis=0)
    y1 = np.concatenate([res.results[i]["out1"] for i in range(NCORES)], axis=0)
    return (y0.astype(np.float32), y1.astype(np.float32)), res


def kernel(**inputs):
    outs, _ = run(inputs)
    return outs

